# revision 11
# baseline (speedup 1.0000x reference)
"""Trainium2 Bass kernel for CSPFM-style pooled channel-attention broadcast.

Math (per batch b):
    d = max(x[b], spatial)                       # [C]
    e = mean(x[b], spatial)                      # [C]
    z = d outer d + e outer e                    # [C, C]
    y = softmax(z, axis=-1)  (no max-subtract: z <= ~25, exp fits fp32)
    f = alpha * (d @ y) + beta * (e @ y)         # [C]
    out[b, c, :, :] = f[c]

Sharding: data-parallel over batch across 8 NeuronCores (4 batches/core).

Per-core schedule (HBM floor ~158us at the ~425GB/s combined DMA rate):
  - 16x 2-MiB input reads as SWDGE cast-DMAs (fp32 HBM -> fp16 SBUF) on the
    gpsimd queue, all issued at kernel start; every tile stays resident so
    reads stream at full rate regardless of compute.
  - pooling: DVE reduce_max + ACT copy-accum per 128-channel chunk.
  - the per-batch stats tile pool has bufs=2, which makes batch b+2's
    pooling wait for batch b's attention chain: this forces the compile-time
    scheduler to interleave [pool b | chain b | pool b+1 | ...] in each
    engine's stream so output writes overlap the remaining reads (without
    it, the scheduler orders all read-gated pooling first and the kernel
    degenerates to reads-then-writes).
  - attention: one PE transpose of the [128,8] stats, one PSUM->SBUF copy,
    one SBUF reshape DMA (on the sync ring, NOT the gpsimd ring where it
    would FIFO behind the input reads) to a [2,C] row tile, one K=2 matmul
    per row chunk (d outer d + e outer e in a single accumulation),
    exp+rowsum on ACT, f = alpha*(hd@E) + beta*(he@E) as [1,C] PSUM rows.
  - broadcast: per chunk, a [128,REP] constant tile is written to the 2-MiB
    output block with a stride-0 (repeat) source AP, so engines only produce
    REP columns instead of 4096.
"""

import os
import sys
from contextlib import ExitStack

import numpy as np

for _p in (
    "/opt/trn_rl_repo",
    "/root/.axon_site",
    "/root/.axon_site/_ro/trn_rl_repo",
    "/root/.axon_site/_ro/pypackages",
):
    if os.path.isdir(_p) and _p not in sys.path:
        sys.path.append(_p)

import concourse.bass as bass  # noqa: E402
import concourse.tile as tile  # noqa: E402
from concourse import bacc, masks, mybir  # noqa: E402
from concourse.bass_utils import run_bass_kernel_spmd  # noqa: E402

F32 = mybir.dt.float32
F16 = mybir.dt.float16
AX = mybir.AxisListType.X
AF = mybir.ActivationFunctionType
MUL = mybir.AluOpType.mult

B, C, H, W = 32, 512, 64, 64
S = H * W                # 4096 spatial positions
NCORES = 8
BL = B // NCORES         # 4 batches per core
NCH = C // 128           # 4 channel chunks of 128
REP = 1024               # broadcast source width (4KB descriptors)
NREP = S // REP


def _emit(tc, out, x, alpha, beta):
    nc = tc.nc
    with ExitStack() as ctx:
        const = ctx.enter_context(tc.tile_pool(name="const", bufs=1))
        xpool = ctx.enter_context(tc.tile_pool(name="xin", bufs=15))
        dpool = ctx.enter_context(tc.tile_pool(name="stats", bufs=2))
        epool = ctx.enter_context(tc.tile_pool(name="expt", bufs=8))
        rpool = ctx.enter_context(tc.tile_pool(name="rep", bufs=4))
        small = ctx.enter_context(tc.tile_pool(name="small", bufs=2))
        mtp = ctx.enter_context(tc.tile_pool(name="mt", bufs=2))
        zpsum = ctx.enter_context(tc.tile_pool(name="zp", bufs=2, space="PSUM"))
        tpsum = ctx.enter_context(tc.tile_pool(name="tp", bufs=1, space="PSUM"))
        fpsum = ctx.enter_context(tc.tile_pool(name="fp", bufs=1, space="PSUM"))
        cpsum = ctx.enter_context(tc.tile_pool(name="cp", bufs=2, space="PSUM"))

        ident = const.tile([128, 128], F32)
        masks.make_identity(nc, ident[:])
        ones1 = const.tile([1, 1], F32)
        nc.vector.memset(ones1[:], 1.0)
        zeros = const.tile([128, REP], F32)
        nc.vector.memset(zeros[:], 0.0)
        # scratch sink for the scalar-engine pooling sums (never read)
        trash = const.tile([128, S // 2], F16)
        ab = const.tile([1, 2], F32)
        nc.sync.dma_start(ab[0:1, 0:1], alpha[:])
        nc.sync.dma_start(ab[0:1, 1:2], beta[:])

        # ---- all input reads up front: fp32 HBM -> fp16 SBUF casts ----
        xts = []
        for b in range(BL):
            row = []
            for cc in range(NCH):
                xt = xpool.tile([128, S], F16)
                nc.gpsimd.dma_start(xt[:], x[b, cc * 128:(cc + 1) * 128, :])
                row.append(xt)
            xts.append(row)

        for b in range(BL):
            # ---- pooling: de8 cols 0..3 = chunk maxes, 4..7 = chunk sums --
            # max: 2-stage fp16 tensor_max tree (TT gets the 2x packed DVE
            # mode; a straight 4096-wide reduce_max runs 1x) + short reduce.
            # sum: ACT copy-accum for 2 chunks, GpSimd reduce for the other
            # 2, so no single engine is slower than the 2-MiB tile arrival.
            de8 = dpool.tile([128, 2 * NCH], F32)
            for cc in range(NCH):
                xt = xts[b][cc]
                m1 = mtp.tile([128, S // 2], F16)
                nc.vector.tensor_max(m1[:], xt[:, 0:S // 2], xt[:, S // 2:S])
                m2 = mtp.tile([128, S // 4], F16)
                nc.vector.tensor_max(m2[:], m1[:, 0:S // 4], m1[:, S // 4:])
                nc.vector.reduce_max(de8[:, cc:cc + 1], m2[:], axis=AX)
                s1 = mtp.tile([128, S // 2], F16)
                nc.gpsimd.tensor_add(s1[:], xt[:, 0:S // 2], xt[:, S // 2:S])
                nc.scalar.activation(
                    trash[:], s1[:], AF.Copy,
                    accum_out=de8[:, NCH + cc:NCH + cc + 1],
                )

            # ---- stats to row layout: t2 row0 = d, row1 = e (mean) ----
            # fold the sum->mean 1/S scale into the stat columns first so
            # every engine op below starts at partition 0
            nc.scalar.mul(de8[:, NCH:2 * NCH], de8[:, NCH:2 * NCH], 1.0 / S)
            tp8 = tpsum.tile([2 * NCH, 128], F32)
            nc.tensor.transpose(tp8[:], de8[:], ident[:])
            t8 = small.tile([2 * NCH, 128], F32)
            nc.vector.tensor_copy(t8[:], tp8[:])
            # NB: must NOT go on the gpsimd queue — it would FIFO behind the
            # remaining multi-MiB input reads and serialize the whole kernel
            t2 = small.tile([2, C], F32)
            nc.sync.dma_start(t2[:], t8[:])

            # ---- z chunk = d^T d + e^T e in ONE K=2 matmul; E = exp(z) ----
            ssum = small.tile([128, NCH], F32)
            rs = small.tile([128, NCH], F32)
            hd = small.tile([128, NCH], F32)
            he = small.tile([128, NCH], F32)
            e_tiles = []
            for ic in range(NCH):
                zp = zpsum.tile([128, C], F32)
                nc.tensor.matmul(zp[:], t2[:, ic * 128:(ic + 1) * 128],
                                 t2[:, 0:C], start=True, stop=True)
                et = epool.tile([128, C], F32)
                nc.scalar.activation(et[:], zp[:], AF.Exp,
                                     accum_out=ssum[:, ic:ic + 1])
                nc.vector.reciprocal(rs[:, ic:ic + 1], ssum[:, ic:ic + 1])
                nc.vector.tensor_mul(hd[:, ic:ic + 1], de8[:, ic:ic + 1],
                                     rs[:, ic:ic + 1])
                nc.vector.tensor_mul(he[:, ic:ic + 1],
                                     de8[:, NCH + ic:NCH + ic + 1],
                                     rs[:, ic:ic + 1])
                e_tiles.append(et)

            # ---- f rows: fmax = hd @ E, fmean = he @ E  (PSUM [1, C]) ----
            fmax = fpsum.tile([1, C], F32)
            fmean = fpsum.tile([1, C], F32)
            for ic in range(NCH):
                nc.tensor.matmul(fmax[:], hd[:, ic:ic + 1], e_tiles[ic][:],
                                 start=(ic == 0), stop=(ic == NCH - 1))
            for ic in range(NCH):
                nc.tensor.matmul(fmean[:], he[:, ic:ic + 1], e_tiles[ic][:],
                                 start=(ic == 0), stop=(ic == NCH - 1))
            t1 = small.tile([1, C], F32)
            nc.vector.tensor_scalar_mul(t1[:], fmax[:], ab[0:1, 0:1])
            t2s = small.tile([1, C], F32)
            nc.scalar.mul(t2s[:], fmean[:], ab[0:1, 1:2])
            frow = small.tile([1, C], F32)
            nc.vector.tensor_add(frow[:], t1[:], t2s[:])

            # ---- per chunk: f column, REP-wide tile, repeat-AP 2-MiB DMA --
            fcs = small.tile([128, NCH], F32)
            for jc in range(NCH):
                fc = cpsum.tile([128, 1], F32)
                nc.tensor.matmul(fc[:], frow[0:1, jc * 128:(jc + 1) * 128],
                                 ones1[:], start=True, stop=True)
                rep = rpool.tile([128, REP], F32)
                if jc % 2 == 0:
                    nc.vector.tensor_copy(fcs[:, jc:jc + 1], fc[:])
                    nc.vector.tensor_scalar_add(rep[:], zeros[:],
                                                fcs[:, jc:jc + 1])
                else:
                    nc.scalar.copy(fcs[:, jc:jc + 1], fc[:])
                    nc.scalar.activation(rep[:], zeros[:], AF.Identity,
                                         bias=fcs[:, jc:jc + 1], scale=1.0)
                eng = nc.sync if jc < 2 else nc.scalar
                eng.dma_start(
                    out[b, jc * 128:(jc + 1) * 128, :],
                    rep[:, None, :].broadcast_to([128, NREP, REP]),
                )


_CACHE = {}
LAST_RESULTS = None


def _build():
    nc = bacc.Bacc("TRN2", target_bir_lowering=False, debug=False,
                   enable_asserts=False, num_devices=NCORES)
    x = nc.dram_tensor("x", [BL, C, S], F32, kind="ExternalInput").ap()
    alpha = nc.dram_tensor("alpha", [1], F32, kind="ExternalInput").ap()
    beta = nc.dram_tensor("beta", [1], F32, kind="ExternalInput").ap()
    out = nc.dram_tensor("out", [BL, C, S], F32, kind="ExternalOutput").ap()
    with tile.TileContext(nc) as tc:
        _emit(tc, out, x, alpha, beta)
    nc.compile()
    return nc


def kernel(x, alpha, beta, _trace=False):
    global LAST_RESULTS
    if "nc" not in _CACHE:
        _CACHE["nc"] = _build()
    nc = _CACHE["nc"]

    xs = np.ascontiguousarray(np.asarray(x, dtype=np.float32).reshape(B, C, S))
    a = np.ascontiguousarray(np.asarray(alpha, dtype=np.float32).reshape(1))
    bt = np.ascontiguousarray(np.asarray(beta, dtype=np.float32).reshape(1))
    in_maps = [
        {"x": xs[k * BL:(k + 1) * BL], "alpha": a, "beta": bt}
        for k in range(NCORES)
    ]
    res = run_bass_kernel_spmd(nc, in_maps, list(range(NCORES)), trace=_trace)
    LAST_RESULTS = res
    full = np.concatenate(
        [np.asarray(res.results[k]["out"]) for k in range(NCORES)], axis=0
    )
    return full.reshape(B, C, H, W).astype(np.float32, copy=False)


# revision 12
# speedup vs baseline: 1.2218x; 1.2218x over previous
"""Trainium2 Bass kernel for CSPFM-style pooled channel-attention broadcast.

Math (per batch b):
    d = max(x[b], spatial)                       # [C]
    e = mean(x[b], spatial)                      # [C]
    z = d outer d + e outer e                    # [C, C]
    y = softmax(z, axis=-1)  (no max-subtract: z <= ~25, exp fits fp32)
    f = alpha * (d @ y) + beta * (e @ y)         # [C]
    out[b, c, :, :] = f[c]

Sharding: data-parallel over batch across 8 NeuronCores (4 batches/core).

Per-core schedule (HBM floor ~158us at the ~425GB/s combined DMA rate):
  - 16x 2-MiB input reads as SWDGE cast-DMAs (fp32 HBM -> fp16 SBUF) on the
    gpsimd queue, all issued at kernel start; every tile stays resident so
    reads stream at full rate regardless of compute.
  - pooling: DVE reduce_max + ACT copy-accum per 128-channel chunk.
  - the per-batch stats tile pool has bufs=2, which makes batch b+2's
    pooling wait for batch b's attention chain: this forces the compile-time
    scheduler to interleave [pool b | chain b | pool b+1 | ...] in each
    engine's stream so output writes overlap the remaining reads (without
    it, the scheduler orders all read-gated pooling first and the kernel
    degenerates to reads-then-writes).
  - attention: one PE transpose of the [128,8] stats, one PSUM->SBUF copy,
    one SBUF reshape DMA (on the sync ring, NOT the gpsimd ring where it
    would FIFO behind the input reads) to a [2,C] row tile, one K=2 matmul
    per row chunk (d outer d + e outer e in a single accumulation),
    exp+rowsum on ACT, f = alpha*(hd@E) + beta*(he@E) as [1,C] PSUM rows.
  - broadcast: per chunk, a [128,REP] constant tile is written to the 2-MiB
    output block with a stride-0 (repeat) source AP, so engines only produce
    REP columns instead of 4096.
"""

import os
import sys
from contextlib import ExitStack

import numpy as np

for _p in (
    "/opt/trn_rl_repo",
    "/root/.axon_site",
    "/root/.axon_site/_ro/trn_rl_repo",
    "/root/.axon_site/_ro/pypackages",
):
    if os.path.isdir(_p) and _p not in sys.path:
        sys.path.append(_p)

import concourse.bass as bass  # noqa: E402
import concourse.tile as tile  # noqa: E402
from concourse import bacc, masks, mybir  # noqa: E402
from concourse.bass_utils import run_bass_kernel_spmd  # noqa: E402

F32 = mybir.dt.float32
F16 = mybir.dt.float16
AX = mybir.AxisListType.X
AF = mybir.ActivationFunctionType
MUL = mybir.AluOpType.mult

B, C, H, W = 32, 512, 64, 64
S = H * W                # 4096 spatial positions
NCORES = 8
BL = B // NCORES         # 4 batches per core
NCH = C // 128           # 4 channel chunks of 128
REP = 1024               # broadcast source width (4KB descriptors)
NREP = S // REP


def _emit(tc, out, x, alpha, beta):
    nc = tc.nc
    with ExitStack() as ctx:
        const = ctx.enter_context(tc.tile_pool(name="const", bufs=1))
        xpool = ctx.enter_context(tc.tile_pool(name="xin", bufs=6))
        dpool = ctx.enter_context(tc.tile_pool(name="stats", bufs=2))
        epool = ctx.enter_context(tc.tile_pool(name="expt", bufs=8))
        rpool = ctx.enter_context(tc.tile_pool(name="rep", bufs=6))
        small = ctx.enter_context(tc.tile_pool(name="small", bufs=2))
        mtp = ctx.enter_context(tc.tile_pool(name="mt", bufs=2))
        zpsum = ctx.enter_context(tc.tile_pool(name="zp", bufs=2, space="PSUM"))
        tpsum = ctx.enter_context(tc.tile_pool(name="tp", bufs=1, space="PSUM"))
        fpsum = ctx.enter_context(tc.tile_pool(name="fp", bufs=1, space="PSUM"))
        cpsum = ctx.enter_context(tc.tile_pool(name="cp", bufs=2, space="PSUM"))

        ident = const.tile([128, 128], F32)
        masks.make_identity(nc, ident[:])
        ones1 = const.tile([1, 1], F32)
        nc.vector.memset(ones1[:], 1.0)
        zeros = const.tile([128, REP], F32)
        nc.vector.memset(zeros[:], 0.0)
        # scratch sink for the scalar-engine pooling sums (never read)
        trash = const.tile([128, S], F16)
        ab = const.tile([1, 2], F32)
        nc.sync.dma_start(ab[0:1, 0:1], alpha[:])
        nc.sync.dma_start(ab[0:1, 1:2], beta[:])

        # ---- all input reads up front: fp32 HBM -> fp16 SBUF casts ----
        xts = []
        for b in range(BL):
            row = []
            for cc in range(NCH):
                xt = xpool.tile([128, S], F16)
                nc.gpsimd.dma_start(xt[:], x[b, cc * 128:(cc + 1) * 128, :])
                row.append(xt)
            xts.append(row)

        for b in range(BL):
            # ---- pooling: de8 cols 0..3 = chunk maxes, 4..7 = chunk sums --
            # max: 2-stage fp16 tensor_max tree (TT gets the 2x packed DVE
            # mode; a straight 4096-wide reduce_max runs 1x) + short reduce.
            # sum: ACT copy-accum (full width). The xpool bufs=6 paces the
            # reads to the engines' consumption rate so the write stream
            # gets HBM bandwidth throughout instead of after all reads.
            de8 = dpool.tile([128, 2 * NCH], F32)
            for cc in range(NCH):
                xt = xts[b][cc]
                m1 = mtp.tile([128, S // 2], F16)
                nc.vector.tensor_max(m1[:], xt[:, 0:S // 2], xt[:, S // 2:S])
                m2 = mtp.tile([128, S // 4], F16)
                nc.vector.tensor_max(m2[:], m1[:, 0:S // 4], m1[:, S // 4:])
                nc.vector.reduce_max(de8[:, cc:cc + 1], m2[:], axis=AX)
                nc.scalar.activation(
                    trash[:], xt[:], AF.Copy,
                    accum_out=de8[:, NCH + cc:NCH + cc + 1],
                )

            # ---- stats to row layout: t2 row0 = d, row1 = e (mean) ----
            # fold the sum->mean 1/S scale into the stat columns first so
            # every engine op below starts at partition 0
            nc.scalar.mul(de8[:, NCH:2 * NCH], de8[:, NCH:2 * NCH], 1.0 / S)
            tp8 = tpsum.tile([2 * NCH, 128], F32)
            nc.tensor.transpose(tp8[:], de8[:], ident[:])
            t8 = small.tile([2 * NCH, 128], F32)
            nc.vector.tensor_copy(t8[:], tp8[:])
            # NB: must NOT go on the gpsimd queue — it would FIFO behind the
            # remaining multi-MiB input reads and serialize the whole kernel
            t2 = small.tile([2, C], F32)
            nc.sync.dma_start(t2[:], t8[:])

            # ---- z chunk = d^T d + e^T e in ONE K=2 matmul; E = exp(z) ----
            ssum = small.tile([128, NCH], F32)
            rs = small.tile([128, NCH], F32)
            hd = small.tile([128, NCH], F32)
            he = small.tile([128, NCH], F32)
            e_tiles = []
            for ic in range(NCH):
                zp = zpsum.tile([128, C], F32)
                nc.tensor.matmul(zp[:], t2[:, ic * 128:(ic + 1) * 128],
                                 t2[:, 0:C], start=True, stop=True)
                et = epool.tile([128, C], F32)
                nc.scalar.activation(et[:], zp[:], AF.Exp,
                                     accum_out=ssum[:, ic:ic + 1])
                nc.vector.reciprocal(rs[:, ic:ic + 1], ssum[:, ic:ic + 1])
                nc.vector.tensor_mul(hd[:, ic:ic + 1], de8[:, ic:ic + 1],
                                     rs[:, ic:ic + 1])
                nc.vector.tensor_mul(he[:, ic:ic + 1],
                                     de8[:, NCH + ic:NCH + ic + 1],
                                     rs[:, ic:ic + 1])
                e_tiles.append(et)

            # ---- f rows: fmax = hd @ E, fmean = he @ E  (PSUM [1, C]) ----
            fmax = fpsum.tile([1, C], F32)
            fmean = fpsum.tile([1, C], F32)
            for ic in range(NCH):
                nc.tensor.matmul(fmax[:], hd[:, ic:ic + 1], e_tiles[ic][:],
                                 start=(ic == 0), stop=(ic == NCH - 1))
            for ic in range(NCH):
                nc.tensor.matmul(fmean[:], he[:, ic:ic + 1], e_tiles[ic][:],
                                 start=(ic == 0), stop=(ic == NCH - 1))
            t1 = small.tile([1, C], F32)
            nc.vector.tensor_scalar_mul(t1[:], fmax[:], ab[0:1, 0:1])
            t2s = small.tile([1, C], F32)
            nc.scalar.mul(t2s[:], fmean[:], ab[0:1, 1:2])
            frow = small.tile([1, C], F32)
            nc.vector.tensor_add(frow[:], t1[:], t2s[:])

            # ---- per chunk: f column, REP-wide tile, repeat-AP 2-MiB DMA --
            fcs = small.tile([128, NCH], F32)
            for jc in range(NCH):
                fc = cpsum.tile([128, 1], F32)
                nc.tensor.matmul(fc[:], frow[0:1, jc * 128:(jc + 1) * 128],
                                 ones1[:], start=True, stop=True)
                rep = rpool.tile([128, REP], F32)
                if jc % 2 == 0:
                    nc.vector.tensor_copy(fcs[:, jc:jc + 1], fc[:])
                    nc.vector.tensor_scalar_add(rep[:], zeros[:],
                                                fcs[:, jc:jc + 1])
                else:
                    nc.scalar.copy(fcs[:, jc:jc + 1], fc[:])
                    nc.scalar.activation(rep[:], zeros[:], AF.Identity,
                                         bias=fcs[:, jc:jc + 1], scale=1.0)
                eng = nc.sync if jc < 2 else nc.scalar
                eng.dma_start(
                    out[b, jc * 128:(jc + 1) * 128, :],
                    rep[:, None, :].broadcast_to([128, NREP, REP]),
                )


_CACHE = {}
LAST_RESULTS = None


def _build():
    nc = bacc.Bacc("TRN2", target_bir_lowering=False, debug=False,
                   enable_asserts=False, num_devices=NCORES)
    x = nc.dram_tensor("x", [BL, C, S], F32, kind="ExternalInput").ap()
    alpha = nc.dram_tensor("alpha", [1], F32, kind="ExternalInput").ap()
    beta = nc.dram_tensor("beta", [1], F32, kind="ExternalInput").ap()
    out = nc.dram_tensor("out", [BL, C, S], F32, kind="ExternalOutput").ap()
    with tile.TileContext(nc) as tc:
        _emit(tc, out, x, alpha, beta)
    nc.compile()
    return nc


def kernel(x, alpha, beta, _trace=False):
    global LAST_RESULTS
    if "nc" not in _CACHE:
        _CACHE["nc"] = _build()
    nc = _CACHE["nc"]

    xs = np.ascontiguousarray(np.asarray(x, dtype=np.float32).reshape(B, C, S))
    a = np.ascontiguousarray(np.asarray(alpha, dtype=np.float32).reshape(1))
    bt = np.ascontiguousarray(np.asarray(beta, dtype=np.float32).reshape(1))
    in_maps = [
        {"x": xs[k * BL:(k + 1) * BL], "alpha": a, "beta": bt}
        for k in range(NCORES)
    ]
    res = run_bass_kernel_spmd(nc, in_maps, list(range(NCORES)), trace=_trace)
    LAST_RESULTS = res
    full = np.concatenate(
        [np.asarray(res.results[k]["out"]) for k in range(NCORES)], axis=0
    )
    return full.reshape(B, C, H, W).astype(np.float32, copy=False)
